# revision 4
# baseline (speedup 1.0000x reference)
"""Trainium2 Bass kernel for nn_BestDetectorEverLoss.

Data-parallel over the batch dim N=65536 across 8 NeuronCores. Each core
streams its 8192 samples (samples on SBUF partitions), computes per-sample
matching / IoU / loss terms, and reduces to per-partition partial sums.
The host combines the 8 cores' partials in float64.

Algorithm notes (validated against the reference in numpy):
  - The argmax grid-cell is selected with an equality mask against the
    row max (no ties on this data), and all gathers are mask-multiply +
    reduce along the 49 cells.
  - The (cx + j)/G, (cy + i)/G translation in the reference cancels in
    every IoU difference, so i/j are never needed.
  - Anchor argmax uses first-match tie-breaking (exact IoU ties do occur
    when several anchors have zero overlap).
  - prob_loss decomposes into a global sum of -ln(1-p) over all anchor
    maps plus a correction sum(probs * (ln(1-p_best) - ln(p_best))).
"""

import numpy as np

N_CORES = 8
N = 65536
G = 7
NC_SAMP = N // N_CORES          # 8192 samples per core
TILE = 1024                     # samples per macro-tile
B = TILE // 128                 # sample groups (free-dim batch) per macro-tile
MT = NC_SAMP // TILE            # macro-tiles per core
N_ACC = 5                       # ce, coord, size, obj, s

_compiled = None


def _split_multi_waits(nc):
    """This walrus build caps sync waits at 1 per instruction (2 for
    EventSemaphore), but Tile's sem assignment can attach several. Hoist
    extra waits onto same-engine NoOps inserted right before the
    instruction — identical blocking semantics, encodable."""
    import bass_rust

    def cap(inst):
        return 2 if isinstance(inst, bass_rust.InstEventSemaphore) else 1

    for f in nc.m.functions:
        for bb in f.blocks:
            il = bb.instructions
            i = 0
            while i < len(il):
                inst = il[i]
                si = getattr(inst, "sync_info", None)
                if si is not None and si.on_wait:
                    k = cap(inst)
                    waits = list(si.on_wait)
                    if len(waits) > k:
                        si.on_wait = waits[:k]
                        for w in waits[k:]:
                            nop = bass_rust.InstNoOp(
                                name=f"nopw-{nc.next_id()}", ins=[], outs=[])
                            nop.engine = inst.engine
                            nop.sync_info = bass_rust.SyncInfo(
                                on_wait=[w], on_update=[])
                            il.insert(i, nop)
                            i += 1
                i += 1


def _build():
    from concourse import bass, mybir
    from concourse.tile import TileContext

    f32 = mybir.dt.float32
    Alu = mybir.AluOpType
    Act = mybir.ActivationFunctionType

    nc = bass.Bass("TRN2", target_bir_lowering=False, debug=False,
                   num_devices=N_CORES)

    coords_d = nc.dram_tensor("coords", [NC_SAMP, 16 * 49], f32,
                              kind="ExternalInput").ap()
    probs_d = nc.dram_tensor("probs", [NC_SAMP, 49], f32,
                             kind="ExternalInput").ap()
    obj_d = nc.dram_tensor("obj", [NC_SAMP, 3 * 49], f32,
                           kind="ExternalInput").ap()
    clsz_d = nc.dram_tensor("clsz", [NC_SAMP, 4], f32,
                            kind="ExternalInput").ap()
    out_d = nc.dram_tensor("out", [128, N_ACC * MT], f32,
                           kind="ExternalOutput").ap()

    with TileContext(nc) as tc:
        with tc.tile_pool(name="const", bufs=1) as cpool, \
             tc.tile_pool(name="acc", bufs=1) as apool, \
             tc.tile_pool(name="io", bufs=2) as io, \
             tc.tile_pool(name="wk", bufs=2) as wk:

            ones = cpool.tile([128, 1], f32)
            nc.vector.memset(ones[:], 1.0)
            revk_i = cpool.tile([128, 3], mybir.dt.int32)
            nc.gpsimd.iota(revk_i[:], pattern=[[-1, 3]], base=2,
                           channel_multiplier=0)
            revk = cpool.tile([128, 3], f32)
            nc.vector.tensor_copy(revk[:], revk_i[:])

            acc = apool.tile([128, N_ACC * MT], f32)

            for mt in range(MT):
                s0 = mt * TILE
                c_t = io.tile([128, B, 16, 49], f32)
                p_t = io.tile([128, B, 49], f32)
                o_t = io.tile([128, B, 3, 49], f32)
                z_t = io.tile([128, B, 4], f32)

                nc.sync.dma_start(
                    out=c_t[:],
                    in_=coords_d[s0:s0 + TILE].rearrange(
                        "(g p) (c k) -> p g c k", p=128, c=16))
                nc.sync.dma_start(
                    out=p_t[:],
                    in_=probs_d[s0:s0 + TILE].rearrange(
                        "(g p) k -> p g k", p=128))
                nc.sync.dma_start(
                    out=o_t[:],
                    in_=obj_d[s0:s0 + TILE].rearrange(
                        "(g p) (a k) -> p g a k", p=128, a=3))
                nc.sync.dma_start(
                    out=z_t[:],
                    in_=clsz_d[s0:s0 + TILE].rearrange(
                        "(g p) c -> p g c", p=128))

                # --- argmax cell one-hot ----------------------------------
                maxv = wk.tile([128, B], f32)
                nc.vector.reduce_max(maxv[:], p_t[:], axis=mybir.AxisListType.X)
                eqm = wk.tile([128, B, 49], f32)
                nc.vector.tensor_tensor(
                    eqm[:], p_t[:],
                    maxv[:].unsqueeze(2).broadcast_to([128, B, 49]),
                    op=Alu.is_equal)

                # --- gather 16 channels at the argmax cell ----------------
                prod = wk.tile([128, B, 16, 49], f32)
                nc.vector.tensor_tensor(
                    prod[:], c_t[:],
                    eqm[:].unsqueeze(2).broadcast_to([128, B, 16, 49]),
                    op=Alu.mult)
                g = wk.tile([128, B, 16], f32)
                nc.vector.reduce_sum(g[:], prod[:], axis=mybir.AxisListType.X)
                # g layout per group: [gx gy gw gh a0x a0y a0w a0h a1... a2h]
                gv = g[:]                                  # [128,B,16]
                g4 = gv.rearrange("p b (x c) -> p b x c", x=4)  # [128,B,4box,4comp]

                # --- IoU (translation-invariant form) ---------------------
                c7 = wk.tile([128, B, 4, 2], f32)
                nc.vector.tensor_scalar_mul(c7[:], g4[:, :, :, 0:2], 1.0 / G)
                wh2 = wk.tile([128, B, 4, 2], f32)
                nc.vector.tensor_scalar_mul(wh2[:], g4[:, :, :, 2:4], 0.5)
                lo = wk.tile([128, B, 4, 2], f32)
                nc.vector.tensor_sub(lo[:], c7[:], wh2[:])
                hi = wk.tile([128, B, 4, 2], f32)
                nc.vector.tensor_add(hi[:], c7[:], wh2[:])

                minhi = wk.tile([128, B, 3, 2], f32)
                nc.vector.tensor_tensor(
                    minhi[:], hi[:, :, 1:4, :],
                    hi[:, :, 0:1, :].broadcast_to([128, B, 3, 2]), op=Alu.min)
                maxlo = wk.tile([128, B, 3, 2], f32)
                nc.vector.tensor_tensor(
                    maxlo[:], lo[:, :, 1:4, :],
                    lo[:, :, 0:1, :].broadcast_to([128, B, 3, 2]), op=Alu.max)
                iwh = wk.tile([128, B, 3, 2], f32)
                nc.vector.tensor_sub(iwh[:], minhi[:], maxlo[:])
                nc.vector.tensor_scalar_max(iwh[:], iwh[:], 0.0)

                inter = wk.tile([128, B, 3], f32)
                nc.vector.tensor_mul(inter[:], iwh[:, :, :, 0], iwh[:, :, :, 1])
                area = wk.tile([128, B, 4], f32)
                nc.vector.tensor_mul(area[:], g4[:, :, :, 2], g4[:, :, :, 3])
                den = wk.tile([128, B, 3], f32)
                nc.vector.tensor_tensor(
                    den[:], area[:, :, 1:4],
                    area[:, :, 0:1].broadcast_to([128, B, 3]), op=Alu.add)
                # den = (area_a + area_t) - inter + 1e-9
                nc.vector.scalar_tensor_tensor(
                    den[:], inter[:], -1.0, den[:],
                    op0=Alu.mult, op1=Alu.add)
                nc.vector.tensor_scalar_add(den[:], den[:], 1e-9)
                rden = wk.tile([128, B, 3], f32)
                nc.vector.reciprocal(rden[:], den[:])
                iou = wk.tile([128, B, 3], f32)
                nc.vector.tensor_mul(iou[:], inter[:], rden[:])

                # --- best anchor, first-match one-hot ---------------------
                bi = wk.tile([128, B], f32)
                nc.vector.reduce_max(bi[:], iou[:], axis=mybir.AxisListType.X)
                eq3 = wk.tile([128, B, 3], f32)
                nc.vector.tensor_tensor(
                    eq3[:], iou[:],
                    bi[:].unsqueeze(2).broadcast_to([128, B, 3]),
                    op=Alu.is_equal)
                mrev = wk.tile([128, B, 3], f32)
                nc.vector.tensor_tensor(
                    mrev[:], eq3[:],
                    revk[:].unsqueeze(1).broadcast_to([128, B, 3]),
                    op=Alu.mult)
                kfm = wk.tile([128, B], f32)
                nc.vector.reduce_max(kfm[:], mrev[:], axis=mybir.AxisListType.X)
                oh3 = wk.tile([128, B, 3], f32)
                nc.vector.tensor_tensor(
                    oh3[:], revk[:].unsqueeze(1).broadcast_to([128, B, 3]),
                    kfm[:].unsqueeze(2).broadcast_to([128, B, 3]),
                    op=Alu.is_equal)

                # --- best box (4 comps) -----------------------------------
                bprod = wk.tile([128, B, 3, 4], f32)
                nc.vector.tensor_tensor(
                    bprod[:], g4[:, :, 1:4, :],
                    oh3[:].unsqueeze(3).broadcast_to([128, B, 3, 4]),
                    op=Alu.mult)
                bb = wk.tile([128, B, 4], f32)
                nc.vector.reduce_sum(bb[:], bprod[:].transpose([0, 1, 3, 2]),
                                     axis=mybir.AxisListType.X)

                # --- selected objectness map ------------------------------
                pprod = wk.tile([128, B, 3, 49], f32)
                nc.vector.tensor_tensor(
                    pprod[:], o_t[:],
                    oh3[:].unsqueeze(3).broadcast_to([128, B, 3, 49]),
                    op=Alu.mult)
                psel = wk.tile([128, B, 49], f32)
                nc.vector.reduce_sum(psel[:], pprod[:].transpose([0, 1, 3, 2]),
                                     axis=mybir.AxisListType.X)

                # --- ACT: logs / exp --------------------------------------
                l0 = wk.tile([128, B, 49], f32)
                nc.scalar.activation(l0[:], psel[:], Act.Ln)
                l1 = wk.tile([128, B, 49], f32)
                nc.scalar.activation(l1[:], psel[:], Act.Ln, bias=1.0, scale=-1.0)
                l1g = wk.tile([128, B, 3, 49], f32)
                nc.scalar.activation(l1g[:], o_t[:], Act.Ln, bias=1.0, scale=-1.0)
                lnbb = wk.tile([128, B, 4], f32)
                nc.scalar.activation(lnbb[:], bb[:], Act.Ln)
                ln1mbb = wk.tile([128, B, 2], f32)
                nc.scalar.activation(ln1mbb[:], bb[:, :, 0:2], Act.Ln,
                                     bias=1.0, scale=-1.0)
                lngt = wk.tile([128, B, 2], f32)
                nc.scalar.activation(lngt[:], g4[:, :, 0, 2:4], Act.Ln)
                expz = wk.tile([128, B, 2], f32)
                nc.scalar.activation(expz[:], z_t[:, :, 0:2], Act.Exp)

                # --- objectness terms -------------------------------------
                d_t = wk.tile([128, B, 49], f32)
                nc.vector.tensor_sub(d_t[:], l1[:], l0[:])
                nc.vector.tensor_mul(d_t[:], d_t[:], p_t[:])
                nc.vector.reduce_sum(acc[:, N_ACC * mt + 4:N_ACC * mt + 5],
                                     d_t[:], axis=mybir.AxisListType.XY)
                nc.vector.reduce_sum(acc[:, N_ACC * mt + 3:N_ACC * mt + 4],
                                     l1g[:], axis=mybir.AxisListType.XYZ)

                # --- coord bce sum ----------------------------------------
                bce = wk.tile([128, B, 2], f32)
                nc.vector.tensor_mul(bce[:], g4[:, :, 0, 0:2], lnbb[:, :, 0:2])
                tc_ = wk.tile([128, B, 2], f32)
                nc.vector.tensor_tensor(
                    tc_[:], ones[:].unsqueeze(2).broadcast_to([128, B, 2]),
                    g4[:, :, 0, 0:2], op=Alu.subtract)
                nc.vector.tensor_mul(tc_[:], tc_[:], ln1mbb[:])
                nc.vector.tensor_add(bce[:], bce[:], tc_[:])
                nc.vector.reduce_sum(acc[:, N_ACC * mt + 1:N_ACC * mt + 2],
                                     bce[:], axis=mybir.AxisListType.XY)

                # --- size term --------------------------------------------
                dsz = wk.tile([128, B, 2], f32)
                nc.vector.tensor_sub(dsz[:], lnbb[:, :, 2:4], lngt[:])
                nc.vector.tensor_reduce(
                    acc[:, N_ACC * mt + 2:N_ACC * mt + 3], dsz[:],
                    axis=mybir.AxisListType.XY, op=Alu.add,
                    apply_absolute_value=True)

                # --- cross-entropy ----------------------------------------
                sez = wk.tile([128, B], f32)
                nc.vector.reduce_sum(sez[:], expz[:], axis=mybir.AxisListType.X)
                lnsez = wk.tile([128, B], f32)
                nc.scalar.activation(lnsez[:], sez[:], Act.Ln)
                ced = wk.tile([128, B], f32)
                nc.vector.tensor_sub(ced[:], z_t[:, :, 1], z_t[:, :, 0])
                nc.vector.tensor_mul(ced[:], ced[:], z_t[:, :, 2])
                nc.vector.tensor_add(ced[:], ced[:], z_t[:, :, 0])
                nc.vector.tensor_sub(ced[:], lnsez[:], ced[:])
                nc.vector.reduce_sum(acc[:, N_ACC * mt:N_ACC * mt + 1],
                                     ced[:], axis=mybir.AxisListType.X)

            nc.sync.dma_start(out=out_d[:], in_=acc[:])

    _split_multi_waits(nc)
    return nc


def _prep_core_inputs(bbox_, bbox, cls_, cls):
    """Shard + pack host-side. Returns list of in_maps for the 8 cores."""
    bbox = np.ascontiguousarray(bbox.reshape(N, 5, 49))
    bbox_ = np.ascontiguousarray(bbox_.reshape(N, 15, 49))
    probs = bbox[:, 0]                                     # [N,49]
    coord_idx = [1, 2, 3, 4, 6, 7, 8, 9, 11, 12, 13, 14]
    coords = np.concatenate(
        [bbox[:, 1:5], bbox_[:, coord_idx]], axis=1).reshape(N, 16 * 49)
    obj = bbox_[:, [0, 5, 10]].reshape(N, 3 * 49)
    clsz = np.zeros((N, 4), np.float32)
    clsz[:, 0:2] = cls_
    clsz[:, 2] = cls.astype(np.float32) - 1.0

    maps = []
    for c in range(N_CORES):
        s = slice(c * NC_SAMP, (c + 1) * NC_SAMP)
        maps.append({
            "coords": np.ascontiguousarray(coords[s]),
            "probs": np.ascontiguousarray(probs[s]),
            "obj": np.ascontiguousarray(obj[s]),
            "clsz": np.ascontiguousarray(clsz[s]),
        })
    return maps


def _combine(results):
    """results: list of per-core dicts with 'out' [128, N_ACC*MT]."""
    parts = np.stack([r["out"] for r in results]).astype(np.float64)
    parts = parts.reshape(N_CORES, 128, MT, N_ACC)
    tot = parts.sum(axis=(0, 1, 2))          # [N_ACC] = ce, coord, size, obj, s
    ce_sum, coord_acc, size_acc, obj_acc, s_acc = tot
    total = ce_sum / N - coord_acc + size_acc + (s_acc - obj_acc) / (N * 49.0)
    return np.float32(total)


def kernel(bbox_, cls_, bbox, cls, _trace=False, _trace_kwargs=None):
    global _compiled
    from concourse.bass_utils import run_bass_kernel_spmd

    bbox_ = np.asarray(bbox_, dtype=np.float32)
    bbox = np.asarray(bbox, dtype=np.float32)
    cls_ = np.asarray(cls_, dtype=np.float32)
    cls = np.asarray(cls)

    if _compiled is None:
        _compiled = _build()
    maps = _prep_core_inputs(bbox_, bbox, cls_, cls)
    kw = {}
    if _trace:
        kw["trace"] = True
        kw.update(_trace_kwargs or {})
    res = run_bass_kernel_spmd(_compiled, maps, list(range(N_CORES)), **kw)
    out = _combine(res.results)
    if _trace:
        return out, res
    return out


# revision 6
# speedup vs baseline: 13.1078x; 13.1078x over previous
"""Trainium2 Bass kernel for nn_BestDetectorEverLoss.

Data-parallel over the batch dim N=65536 across 8 NeuronCores. Each core
streams its 8192 samples (samples on SBUF partitions), computes per-sample
matching / IoU / loss terms, and reduces to per-partition partial sums.
The host combines the 8 cores' partials in float64.

Algorithm notes (validated against the reference in numpy):
  - The argmax grid-cell is selected with an equality mask against the
    row max (no ties on this data), and all gathers are mask-multiply +
    reduce along the 49 cells.
  - The (cx + j)/G, (cy + i)/G translation in the reference cancels in
    every IoU difference, so i/j are never needed.
  - Anchor argmax uses first-match tie-breaking (exact IoU ties do occur
    when several anchors have zero overlap).
  - prob_loss decomposes into a global sum of -ln(1-p) over all anchor
    maps plus a correction sum(probs * (ln(1-p_best) - ln(p_best))).
"""

import numpy as np

N_CORES = 8
N = 65536
G = 7
NC_SAMP = N // N_CORES          # 8192 samples per core
TILE = 1024                     # samples per macro-tile
B = TILE // 128                 # sample groups (free-dim batch) per macro-tile
MT = NC_SAMP // TILE            # macro-tiles per core
N_ACC = 5                       # ce, coord, size, obj, s

_compiled = None


def _split_multi_waits(nc):
    """This walrus build caps sync waits at 1 per instruction (2 for
    EventSemaphore), but Tile's sem assignment can attach several. Hoist
    extra waits onto same-engine NoOps inserted right before the
    instruction — identical blocking semantics, encodable."""
    import bass_rust

    def cap(inst):
        return 2 if isinstance(inst, bass_rust.InstEventSemaphore) else 1

    for f in nc.m.functions:
        for bb in f.blocks:
            il = bb.instructions
            i = 0
            while i < len(il):
                inst = il[i]
                si = getattr(inst, "sync_info", None)
                if si is not None and si.on_wait:
                    k = cap(inst)
                    waits = list(si.on_wait)
                    if len(waits) > k:
                        si.on_wait = waits[:k]
                        for w in waits[k:]:
                            nop = bass_rust.InstNoOp(
                                name=f"nopw-{nc.next_id()}", ins=[], outs=[])
                            nop.engine = inst.engine
                            nop.sync_info = bass_rust.SyncInfo(
                                on_wait=[w], on_update=[])
                            il.insert(i, nop)
                            i += 1
                i += 1


def _build(repeat=1):
    from concourse import bass, mybir
    from concourse.tile import TileContext

    f32 = mybir.dt.float32
    Alu = mybir.AluOpType
    Act = mybir.ActivationFunctionType

    nc = bass.Bass("TRN2", target_bir_lowering=False, debug=False,
                   num_devices=N_CORES)

    coords_d = nc.dram_tensor("coords", [NC_SAMP, 16 * 49], f32,
                              kind="ExternalInput").ap()
    probs_d = nc.dram_tensor("probs", [NC_SAMP, 49], f32,
                             kind="ExternalInput").ap()
    obj_d = nc.dram_tensor("obj", [NC_SAMP, 3 * 49], f32,
                           kind="ExternalInput").ap()
    clsz_d = nc.dram_tensor("clsz", [NC_SAMP, 4], f32,
                            kind="ExternalInput").ap()
    out_d = nc.dram_tensor("out", [128, N_ACC * MT], f32,
                           kind="ExternalOutput").ap()

    with TileContext(nc) as tc:
        with tc.tile_pool(name="const", bufs=1) as cpool, \
             tc.tile_pool(name="acc", bufs=1) as apool, \
             tc.tile_pool(name="io", bufs=2) as io, \
             tc.tile_pool(name="wk", bufs=2) as wk:

            ones = cpool.tile([128, 1], f32)
            nc.vector.memset(ones[:], 1.0)
            revk_i = cpool.tile([128, 3], mybir.dt.int32)
            nc.gpsimd.iota(revk_i[:], pattern=[[-1, 3]], base=2,
                           channel_multiplier=0)
            revk = cpool.tile([128, 3], f32)
            nc.vector.tensor_copy(revk[:], revk_i[:])

            acc = apool.tile([128, N_ACC * MT], f32)

            for mt in [m for _ in range(repeat) for m in range(MT)]:
                s0 = mt * TILE
                c_t = io.tile([128, B, 16, 49], f32)
                p_t = io.tile([128, B, 49], f32)
                o_t = io.tile([128, B, 3, 49], f32)
                z_t = io.tile([128, B, 4], f32)

                nc.sync.dma_start(
                    out=c_t[:],
                    in_=coords_d[s0:s0 + TILE].rearrange(
                        "(g p) (c k) -> p g c k", p=128, c=16))
                nc.sync.dma_start(
                    out=p_t[:],
                    in_=probs_d[s0:s0 + TILE].rearrange(
                        "(g p) k -> p g k", p=128))
                nc.sync.dma_start(
                    out=o_t[:],
                    in_=obj_d[s0:s0 + TILE].rearrange(
                        "(g p) (a k) -> p g a k", p=128, a=3))
                nc.sync.dma_start(
                    out=z_t[:],
                    in_=clsz_d[s0:s0 + TILE].rearrange(
                        "(g p) c -> p g c", p=128))

                # --- argmax cell one-hot ----------------------------------
                maxv = wk.tile([128, B], f32)
                nc.vector.reduce_max(maxv[:], p_t[:], axis=mybir.AxisListType.X)
                eqm = wk.tile([128, B, 49], f32)
                nc.vector.tensor_tensor(
                    eqm[:], p_t[:],
                    maxv[:].unsqueeze(2).broadcast_to([128, B, 49]),
                    op=Alu.is_equal)

                # --- gather 16 channels at the argmax cell ----------------
                prod = wk.tile([128, B, 16, 49], f32)
                nc.vector.tensor_tensor(
                    prod[:], c_t[:],
                    eqm[:].unsqueeze(2).broadcast_to([128, B, 16, 49]),
                    op=Alu.mult)
                g = wk.tile([128, B, 16], f32)
                nc.vector.reduce_sum(g[:], prod[:], axis=mybir.AxisListType.X)
                # g layout per group: [gx gy gw gh a0x a0y a0w a0h a1... a2h]
                gv = g[:]                                  # [128,B,16]
                g4 = gv.rearrange("p b (x c) -> p b x c", x=4)  # [128,B,4box,4comp]

                # --- IoU (translation-invariant form) ---------------------
                c7 = wk.tile([128, B, 4, 2], f32)
                nc.vector.tensor_scalar_mul(c7[:], g4[:, :, :, 0:2], 1.0 / G)
                wh2 = wk.tile([128, B, 4, 2], f32)
                nc.vector.tensor_scalar_mul(wh2[:], g4[:, :, :, 2:4], 0.5)
                lo = wk.tile([128, B, 4, 2], f32)
                nc.vector.tensor_sub(lo[:], c7[:], wh2[:])
                hi = wk.tile([128, B, 4, 2], f32)
                nc.vector.tensor_add(hi[:], c7[:], wh2[:])

                minhi = wk.tile([128, B, 3, 2], f32)
                nc.vector.tensor_tensor(
                    minhi[:], hi[:, :, 1:4, :],
                    hi[:, :, 0:1, :].broadcast_to([128, B, 3, 2]), op=Alu.min)
                maxlo = wk.tile([128, B, 3, 2], f32)
                nc.vector.tensor_tensor(
                    maxlo[:], lo[:, :, 1:4, :],
                    lo[:, :, 0:1, :].broadcast_to([128, B, 3, 2]), op=Alu.max)
                iwh = wk.tile([128, B, 3, 2], f32)
                nc.vector.tensor_sub(iwh[:], minhi[:], maxlo[:])
                nc.vector.tensor_scalar_max(iwh[:], iwh[:], 0.0)

                inter = wk.tile([128, B, 3], f32)
                nc.vector.tensor_mul(inter[:], iwh[:, :, :, 0], iwh[:, :, :, 1])
                area = wk.tile([128, B, 4], f32)
                nc.vector.tensor_mul(area[:], g4[:, :, :, 2], g4[:, :, :, 3])
                den = wk.tile([128, B, 3], f32)
                nc.vector.tensor_tensor(
                    den[:], area[:, :, 1:4],
                    area[:, :, 0:1].broadcast_to([128, B, 3]), op=Alu.add)
                # den = (area_a + area_t) - inter + 1e-9
                nc.vector.scalar_tensor_tensor(
                    den[:], inter[:], -1.0, den[:],
                    op0=Alu.mult, op1=Alu.add)
                nc.vector.tensor_scalar_add(den[:], den[:], 1e-9)
                rden = wk.tile([128, B, 3], f32)
                nc.vector.reciprocal(rden[:], den[:])
                iou = wk.tile([128, B, 3], f32)
                nc.vector.tensor_mul(iou[:], inter[:], rden[:])

                # --- best anchor, first-match one-hot ---------------------
                bi = wk.tile([128, B], f32)
                nc.vector.reduce_max(bi[:], iou[:], axis=mybir.AxisListType.X)
                eq3 = wk.tile([128, B, 3], f32)
                nc.vector.tensor_tensor(
                    eq3[:], iou[:],
                    bi[:].unsqueeze(2).broadcast_to([128, B, 3]),
                    op=Alu.is_equal)
                mrev = wk.tile([128, B, 3], f32)
                nc.vector.tensor_tensor(
                    mrev[:], eq3[:],
                    revk[:].unsqueeze(1).broadcast_to([128, B, 3]),
                    op=Alu.mult)
                kfm = wk.tile([128, B], f32)
                nc.vector.reduce_max(kfm[:], mrev[:], axis=mybir.AxisListType.X)
                oh3 = wk.tile([128, B, 3], f32)
                nc.vector.tensor_tensor(
                    oh3[:], revk[:].unsqueeze(1).broadcast_to([128, B, 3]),
                    kfm[:].unsqueeze(2).broadcast_to([128, B, 3]),
                    op=Alu.is_equal)

                # --- best box (4 comps) -----------------------------------
                bprod = wk.tile([128, B, 3, 4], f32)
                nc.vector.tensor_tensor(
                    bprod[:], g4[:, :, 1:4, :],
                    oh3[:].unsqueeze(3).broadcast_to([128, B, 3, 4]),
                    op=Alu.mult)
                bb = wk.tile([128, B, 4], f32)
                nc.vector.reduce_sum(bb[:], bprod[:].transpose([0, 1, 3, 2]),
                                     axis=mybir.AxisListType.X)

                # --- selected objectness map ------------------------------
                pprod = wk.tile([128, B, 3, 49], f32)
                nc.vector.tensor_tensor(
                    pprod[:], o_t[:],
                    oh3[:].unsqueeze(3).broadcast_to([128, B, 3, 49]),
                    op=Alu.mult)
                psel = wk.tile([128, B, 49], f32)
                nc.vector.reduce_sum(psel[:], pprod[:].transpose([0, 1, 3, 2]),
                                     axis=mybir.AxisListType.X)

                # --- ACT: logs / exp --------------------------------------
                l0 = wk.tile([128, B, 49], f32)
                nc.scalar.activation(l0[:], psel[:], Act.Ln)
                l1 = wk.tile([128, B, 49], f32)
                nc.scalar.activation(l1[:], psel[:], Act.Ln, bias=1.0, scale=-1.0)
                l1g = wk.tile([128, B, 3, 49], f32)
                nc.scalar.activation(l1g[:], o_t[:], Act.Ln, bias=1.0, scale=-1.0)
                lnbb = wk.tile([128, B, 4], f32)
                nc.scalar.activation(lnbb[:], bb[:], Act.Ln)
                ln1mbb = wk.tile([128, B, 2], f32)
                nc.scalar.activation(ln1mbb[:], bb[:, :, 0:2], Act.Ln,
                                     bias=1.0, scale=-1.0)
                lngt = wk.tile([128, B, 2], f32)
                nc.scalar.activation(lngt[:], g4[:, :, 0, 2:4], Act.Ln)
                expz = wk.tile([128, B, 2], f32)
                nc.scalar.activation(expz[:], z_t[:, :, 0:2], Act.Exp)

                # --- objectness terms -------------------------------------
                d_t = wk.tile([128, B, 49], f32)
                nc.vector.tensor_sub(d_t[:], l1[:], l0[:])
                nc.vector.tensor_mul(d_t[:], d_t[:], p_t[:])
                nc.vector.reduce_sum(acc[:, N_ACC * mt + 4:N_ACC * mt + 5],
                                     d_t[:], axis=mybir.AxisListType.XY)
                nc.vector.reduce_sum(acc[:, N_ACC * mt + 3:N_ACC * mt + 4],
                                     l1g[:], axis=mybir.AxisListType.XYZ)

                # --- coord bce sum ----------------------------------------
                bce = wk.tile([128, B, 2], f32)
                nc.vector.tensor_mul(bce[:], g4[:, :, 0, 0:2], lnbb[:, :, 0:2])
                tc_ = wk.tile([128, B, 2], f32)
                nc.vector.tensor_tensor(
                    tc_[:], ones[:].unsqueeze(2).broadcast_to([128, B, 2]),
                    g4[:, :, 0, 0:2], op=Alu.subtract)
                nc.vector.tensor_mul(tc_[:], tc_[:], ln1mbb[:])
                nc.vector.tensor_add(bce[:], bce[:], tc_[:])
                nc.vector.reduce_sum(acc[:, N_ACC * mt + 1:N_ACC * mt + 2],
                                     bce[:], axis=mybir.AxisListType.XY)

                # --- size term --------------------------------------------
                dsz = wk.tile([128, B, 2], f32)
                nc.vector.tensor_sub(dsz[:], lnbb[:, :, 2:4], lngt[:])
                nc.vector.tensor_reduce(
                    acc[:, N_ACC * mt + 2:N_ACC * mt + 3], dsz[:],
                    axis=mybir.AxisListType.XY, op=Alu.add,
                    apply_absolute_value=True)

                # --- cross-entropy ----------------------------------------
                sez = wk.tile([128, B], f32)
                nc.vector.reduce_sum(sez[:], expz[:], axis=mybir.AxisListType.X)
                lnsez = wk.tile([128, B], f32)
                nc.scalar.activation(lnsez[:], sez[:], Act.Ln)
                ced = wk.tile([128, B], f32)
                nc.vector.tensor_sub(ced[:], z_t[:, :, 1], z_t[:, :, 0])
                nc.vector.tensor_mul(ced[:], ced[:], z_t[:, :, 2])
                nc.vector.tensor_add(ced[:], ced[:], z_t[:, :, 0])
                nc.vector.tensor_sub(ced[:], lnsez[:], ced[:])
                nc.vector.reduce_sum(acc[:, N_ACC * mt:N_ACC * mt + 1],
                                     ced[:], axis=mybir.AxisListType.X)

            nc.sync.dma_start(out=out_d[:], in_=acc[:])

    _split_multi_waits(nc)
    return nc


def _prep_core_inputs(bbox_, bbox, cls_, cls):
    """Shard + pack host-side. Returns list of in_maps for the 8 cores."""
    bbox = np.ascontiguousarray(bbox.reshape(N, 5, 49))
    bbox_ = np.ascontiguousarray(bbox_.reshape(N, 15, 49))
    probs = bbox[:, 0]                                     # [N,49]
    coord_idx = [1, 2, 3, 4, 6, 7, 8, 9, 11, 12, 13, 14]
    coords = np.concatenate(
        [bbox[:, 1:5], bbox_[:, coord_idx]], axis=1).reshape(N, 16 * 49)
    obj = bbox_[:, [0, 5, 10]].reshape(N, 3 * 49)
    clsz = np.zeros((N, 4), np.float32)
    clsz[:, 0:2] = cls_
    clsz[:, 2] = cls.astype(np.float32) - 1.0

    maps = []
    for c in range(N_CORES):
        s = slice(c * NC_SAMP, (c + 1) * NC_SAMP)
        maps.append({
            "coords": np.ascontiguousarray(coords[s]),
            "probs": np.ascontiguousarray(probs[s]),
            "obj": np.ascontiguousarray(obj[s]),
            "clsz": np.ascontiguousarray(clsz[s]),
        })
    return maps


def _combine(results):
    """results: list of per-core dicts with 'out' [128, N_ACC*MT]."""
    parts = np.stack([r["out"] for r in results]).astype(np.float64)
    parts = parts.reshape(N_CORES, 128, MT, N_ACC)
    tot = parts.sum(axis=(0, 1, 2))          # [N_ACC] = ce, coord, size, obj, s
    ce_sum, coord_acc, size_acc, obj_acc, s_acc = tot
    total = ce_sum / N - coord_acc + size_acc + (s_acc - obj_acc) / (N * 49.0)
    return np.float32(total)


def kernel(bbox_, cls_, bbox, cls, _trace=False, _trace_kwargs=None):
    global _compiled
    from concourse.bass_utils import run_bass_kernel_spmd

    bbox_ = np.asarray(bbox_, dtype=np.float32)
    bbox = np.asarray(bbox, dtype=np.float32)
    cls_ = np.asarray(cls_, dtype=np.float32)
    cls = np.asarray(cls)

    if _compiled is None:
        _compiled = _build()
    maps = _prep_core_inputs(bbox_, bbox, cls_, cls)
    kw = {}
    if _trace:
        kw["trace"] = True
        kw.update(_trace_kwargs or {})
    res = run_bass_kernel_spmd(_compiled, maps, list(range(N_CORES)), **kw)
    out = _combine(res.results)
    if _trace:
        return out, res
    return out


# revision 10
# speedup vs baseline: 48.3311x; 3.6872x over previous
"""Trainium2 Bass kernel for nn_BestDetectorEverLoss.

Data-parallel over the batch dim N=65536 across 8 NeuronCores. Each core
streams its 8192 samples, computes per-sample matching / IoU / loss terms,
and reduces to per-partition partial sums; the host combines in float64.

v2 design:
  - The 16-value per-sample gather at the argmax cell runs on GpSimd via
    `indirect_copy` over a channels-on-partitions layout (each 16-partition
    group holds all 16 channels of a sample subset; per-sample cell index
    shared by the group's partitions). A PE transpose brings the gathered
    values back to samples-on-partitions.
  - The argmax chain runs in a matching "idx layout" so the uint16 index
    tile is a pure AP transform of the argmax result.
  - coords and objectness maps travel as bfloat16 (the 49-cell argmax uses
    full f32 probs, so the matched cell is exact); everything loss-critical
    is computed in f32 on-chip.
  - (cx+j)/G translation cancels in all IoU differences, so i/j are never
    computed. Anchor argmax uses first-match tie-breaking.
  - prob_loss = [sum -ln(1-p) over all anchor maps]  (ACT accum_out)
              + [sum probs*(ln(1-p_best) - ln(p_best))].
"""

import numpy as np

N_CORES = 8
N = 65536
G = 7
NC_SAMP = N // N_CORES          # 8192 samples per core
QT = 8                          # sample groups ("q") per macro-tile
TILE = 128 * QT                 # samples per macro-tile
MT = NC_SAMP // TILE            # macro-tiles per core
NI = TILE // 8                  # indices per 16-partition gather group
N_ACC = 5                       # ce, coord, size, obj, s

_compiled = None


def _split_multi_waits(nc):
    """This walrus build caps sync waits at 1 per instruction (2 for
    EventSemaphore), but Tile's sem assignment can attach several. Hoist
    extra waits onto same-engine NoOps inserted right before the
    instruction — identical blocking semantics, encodable."""
    import bass_rust

    def cap(inst):
        return 2 if isinstance(inst, bass_rust.InstEventSemaphore) else 1

    for f in nc.m.functions:
        for bb in f.blocks:
            il = bb.instructions
            i = 0
            while i < len(il):
                inst = il[i]
                si = getattr(inst, "sync_info", None)
                if si is not None and si.on_wait:
                    k = cap(inst)
                    waits = list(si.on_wait)
                    if len(waits) > k:
                        si.on_wait = waits[:k]
                        for w in waits[k:]:
                            nop = bass_rust.InstNoOp(
                                name=f"nopw-{nc.next_id()}", ins=[], outs=[])
                            nop.engine = inst.engine
                            nop.sync_info = bass_rust.SyncInfo(
                                on_wait=[w], on_update=[])
                            il.insert(i, nop)
                            i += 1
                i += 1


def _build(repeat=1):
    from concourse import bass, mybir
    from concourse.tile import TileContext

    f32 = mybir.dt.float32
    bf16 = mybir.dt.bfloat16
    u16 = mybir.dt.uint16
    i32 = mybir.dt.int32
    Alu = mybir.AluOpType
    Act = mybir.ActivationFunctionType
    X, XY, XYZ = (mybir.AxisListType.X, mybir.AxisListType.XY,
                  mybir.AxisListType.XYZ)

    nc = bass.Bass("TRN2", target_bir_lowering=False, debug=False,
                   num_devices=N_CORES)

    # DRAM inputs (per-core, host-packed layouts; see _prep_core_inputs)
    cg_d = nc.dram_tensor("cg", [128, MT, NI * 49], bf16,
                          kind="ExternalInput").ap()
    pidx_d = nc.dram_tensor("pidx", [128, MT, QT, 49], f32,
                            kind="ExternalInput").ap()
    pcmp_d = nc.dram_tensor("pcmp", [128, MT, QT, 49], f32,
                            kind="ExternalInput").ap()
    obj_d = nc.dram_tensor("obj", [128, MT, QT, 3, 49], bf16,
                           kind="ExternalInput").ap()
    clsz_d = nc.dram_tensor("clsz", [128, MT, QT, 4], f32,
                            kind="ExternalInput").ap()
    goff_d = nc.dram_tensor("goff", [128, QT], u16,
                            kind="ExternalInput").ap()
    out_d = nc.dram_tensor("out", [128, N_ACC * MT], f32,
                           kind="ExternalOutput").ap()

    with TileContext(nc) as tc:
        with tc.tile_pool(name="const", bufs=1) as cpool, \
             tc.tile_pool(name="acc", bufs=1) as apool, \
             tc.tile_pool(name="io", bufs=2) as io, \
             tc.tile_pool(name="wk", bufs=2) as wk, \
             tc.tile_pool(name="ps", bufs=2, space="PSUM") as psp:

            ones = cpool.tile([128, 1], f32)
            nc.vector.memset(ones[:], 1.0)
            # rev49[c] = 48 - c  (first-match argmax over cells)
            rev49i = cpool.tile([128, 49], i32)
            nc.gpsimd.iota(rev49i[:], pattern=[[-1, 49]], base=48,
                           channel_multiplier=0)
            rev49 = cpool.tile([128, 49], f32)
            nc.vector.tensor_copy(rev49[:], rev49i[:])
            # revk[k] = 2 - k (first-match argmax over anchors)
            revki = cpool.tile([128, 3], i32)
            nc.gpsimd.iota(revki[:], pattern=[[-1, 3]], base=2,
                           channel_multiplier=0)
            revk = cpool.tile([128, 3], f32)
            nc.vector.tensor_copy(revk[:], revki[:])
            # identity (bf16) for PE transpose
            idni = cpool.tile([128, 128], i32)
            nc.gpsimd.iota(idni[:], pattern=[[1, 128]], base=0,
                           channel_multiplier=-1)
            idn = cpool.tile([128, 128], bf16)
            nc.vector.tensor_scalar(idn[:], idni[:], 0, None, op0=Alu.is_equal)
            goff = cpool.tile([128, QT], u16)
            nc.sync.dma_start(out=goff[:], in_=goff_d[:])

            acc = apool.tile([128, N_ACC * MT], f32)

            for mt in [m for _ in range(repeat) for m in range(MT)]:
                a0 = N_ACC * mt
                cg_t = io.tile([128, NI * 49], bf16)
                pidx_t = io.tile([128, QT, 49], f32)
                pcmp_t = io.tile([128, QT, 49], f32)
                o_t = io.tile([128, QT, 3, 49], bf16)
                z_t = io.tile([128, QT, 4], f32)
                nc.sync.dma_start(out=cg_t[:], in_=cg_d[:, mt])
                nc.sync.dma_start(out=pidx_t[:], in_=pidx_d[:, mt])
                nc.sync.dma_start(out=pcmp_t[:], in_=pcmp_d[:, mt])
                nc.sync.dma_start(out=o_t[:], in_=obj_d[:, mt])
                nc.sync.dma_start(out=z_t[:], in_=clsz_d[:, mt])

                # --- argmax cell (idx layout) -----------------------------
                maxv = wk.tile([128, QT], f32)
                nc.vector.reduce_max(maxv[:], pidx_t[:], axis=X)
                eqm = wk.tile([128, QT, 49], f32)
                nc.vector.tensor_tensor(
                    eqm[:], pidx_t[:],
                    maxv[:].unsqueeze(2).broadcast_to([128, QT, 49]),
                    op=Alu.is_equal)
                mrev = wk.tile([128, QT, 49], f32)
                nc.gpsimd.tensor_tensor(
                    mrev[:], eqm[:],
                    rev49[:].unsqueeze(1).broadcast_to([128, QT, 49]),
                    op=Alu.mult)
                mx = wk.tile([128, QT], f32)
                nc.vector.reduce_max(mx[:], mrev[:], axis=X)   # = 48 - m
                mxu = wk.tile([128, QT], u16)
                nc.vector.tensor_copy(mxu[:], mx[:])
                idxs = wk.tile([128, QT], u16)
                nc.vector.tensor_sub(idxs[:], goff[:], mxu[:])  # goff = 49i+48

                # --- gather via indirect_copy + PE transpose --------------
                go = wk.tile([128, NI, 1], bf16)
                nc.gpsimd.indirect_copy(go[:], cg_t[:], idxs[:], True)
                ps = psp.tile([128, NI], bf16)
                nc.tensor.transpose(ps[:], go[:].squeeze(2), idn[:, 0:NI])
                gt = wk.tile([128, NI], f32)
                nc.scalar.copy(gt[:], ps[:])
                # gt[i, 16q+ch]; ch = 4*box + comp; boxes: gt,a0,a1,a2
                g4 = gt[:].rearrange("p (q b c) -> p q b c", b=4, c=4)

                # --- IoU (translation-invariant) --------------------------
                c7 = wk.tile([128, QT, 4, 2], f32)
                nc.vector.tensor_scalar_mul(c7[:], g4[:, :, :, 0:2], 1.0 / G)
                wh2 = wk.tile([128, QT, 4, 2], f32)
                nc.vector.tensor_scalar_mul(wh2[:], g4[:, :, :, 2:4], 0.5)
                lo_ = wk.tile([128, QT, 4, 2], f32)
                nc.vector.tensor_sub(lo_[:], c7[:], wh2[:])
                hi_ = wk.tile([128, QT, 4, 2], f32)
                nc.vector.tensor_add(hi_[:], c7[:], wh2[:])

                minhi = wk.tile([128, QT, 3, 2], f32)
                nc.vector.tensor_tensor(
                    minhi[:], hi_[:, :, 1:4, :],
                    hi_[:, :, 0:1, :].broadcast_to([128, QT, 3, 2]), op=Alu.min)
                maxlo = wk.tile([128, QT, 3, 2], f32)
                nc.vector.tensor_tensor(
                    maxlo[:], lo_[:, :, 1:4, :],
                    lo_[:, :, 0:1, :].broadcast_to([128, QT, 3, 2]), op=Alu.max)
                iwh = wk.tile([128, QT, 3, 2], f32)
                nc.vector.tensor_sub(iwh[:], minhi[:], maxlo[:])
                nc.vector.tensor_scalar_max(iwh[:], iwh[:], 0.0)

                inter = wk.tile([128, QT, 3], f32)
                nc.vector.tensor_mul(inter[:], iwh[:, :, :, 0], iwh[:, :, :, 1])
                area = wk.tile([128, QT, 4], f32)
                nc.vector.tensor_mul(area[:], g4[:, :, :, 2], g4[:, :, :, 3])
                den = wk.tile([128, QT, 3], f32)
                nc.vector.tensor_tensor(
                    den[:], area[:, :, 1:4],
                    area[:, :, 0:1].broadcast_to([128, QT, 3]), op=Alu.add)
                nc.vector.scalar_tensor_tensor(
                    den[:], inter[:], -1.0, den[:], op0=Alu.mult, op1=Alu.add)
                nc.vector.tensor_scalar_add(den[:], den[:], 1e-9)
                rden = wk.tile([128, QT, 3], f32)
                nc.vector.reciprocal(rden[:], den[:])
                iou = wk.tile([128, QT, 3], f32)
                nc.vector.tensor_mul(iou[:], inter[:], rden[:])

                # --- best anchor (first-match one-hot) --------------------
                bi = wk.tile([128, QT], f32)
                nc.vector.reduce_max(bi[:], iou[:], axis=X)
                eq3 = wk.tile([128, QT, 3], f32)
                nc.vector.tensor_tensor(
                    eq3[:], iou[:],
                    bi[:].unsqueeze(2).broadcast_to([128, QT, 3]),
                    op=Alu.is_equal)
                mrev3 = wk.tile([128, QT, 3], f32)
                nc.vector.tensor_tensor(
                    mrev3[:], eq3[:],
                    revk[:].unsqueeze(1).broadcast_to([128, QT, 3]),
                    op=Alu.mult)
                kfm = wk.tile([128, QT], f32)
                nc.vector.reduce_max(kfm[:], mrev3[:], axis=X)
                oh3 = wk.tile([128, QT, 3], f32)
                nc.vector.tensor_tensor(
                    oh3[:], revk[:].unsqueeze(1).broadcast_to([128, QT, 3]),
                    kfm[:].unsqueeze(2).broadcast_to([128, QT, 3]),
                    op=Alu.is_equal)

                # --- best box -------------------------------------------
                bprod = wk.tile([128, QT, 3, 4], f32)
                nc.vector.tensor_tensor(
                    bprod[:], g4[:, :, 1:4, :],
                    oh3[:].unsqueeze(3).broadcast_to([128, QT, 3, 4]),
                    op=Alu.mult)
                bb = wk.tile([128, QT, 4], f32)
                nc.vector.reduce_sum(bb[:], bprod[:].transpose([0, 1, 3, 2]),
                                     axis=X)

                # --- selected objectness map (GpSimd) ---------------------
                msk = wk.tile([128, QT, 3, 49], f32)
                nc.gpsimd.tensor_tensor(
                    msk[:], o_t[:],
                    oh3[:].unsqueeze(3).broadcast_to([128, QT, 3, 49]),
                    op=Alu.mult)
                ps01 = wk.tile([128, QT, 49], f32)
                nc.gpsimd.tensor_tensor(ps01[:], msk[:, :, 0, :],
                                        msk[:, :, 1, :], op=Alu.add)
                psel = wk.tile([128, QT, 49], f32)
                nc.vector.tensor_add(psel[:], ps01[:], msk[:, :, 2, :])

                # --- ACT: logs / exp --------------------------------------
                l0 = wk.tile([128, QT, 49], f32)
                nc.scalar.activation(l0[:], psel[:], Act.Ln)
                l1 = wk.tile([128, QT, 49], f32)
                nc.scalar.activation(l1[:], psel[:], Act.Ln, bias=1.0,
                                     scale=-1.0)
                l1g = wk.tile([128, QT, 3, 49], bf16)
                nc.scalar.activation(l1g[:], o_t[:], Act.Ln, bias=1.0,
                                     scale=-1.0, accum_out=acc[:, a0+3:a0+4])
                lnbb = wk.tile([128, QT, 4], f32)
                nc.scalar.activation(lnbb[:], bb[:], Act.Ln)
                ln1mbb = wk.tile([128, QT, 2], f32)
                nc.scalar.activation(ln1mbb[:], bb[:, :, 0:2], Act.Ln,
                                     bias=1.0, scale=-1.0)
                lngt = wk.tile([128, QT, 2], f32)
                nc.scalar.activation(lngt[:], g4[:, :, 0, 2:4], Act.Ln)
                expz = wk.tile([128, QT, 2], f32)
                nc.scalar.activation(expz[:], z_t[:, :, 0:2], Act.Exp)

                # --- objectness s-term ------------------------------------
                d_t = wk.tile([128, QT, 49], f32)
                nc.gpsimd.tensor_tensor(d_t[:], l1[:], l0[:], op=Alu.subtract)
                nc.gpsimd.tensor_tensor(d_t[:], d_t[:], pcmp_t[:], op=Alu.mult)
                nc.vector.reduce_sum(acc[:, a0+4:a0+5], d_t[:], axis=XY)

                # --- coord bce sum ----------------------------------------
                bce = wk.tile([128, QT, 2], f32)
                nc.vector.tensor_mul(bce[:], g4[:, :, 0, 0:2], lnbb[:, :, 0:2])
                tc_ = wk.tile([128, QT, 2], f32)
                nc.vector.tensor_tensor(
                    tc_[:], ones[:].unsqueeze(2).broadcast_to([128, QT, 2]),
                    g4[:, :, 0, 0:2], op=Alu.subtract)
                nc.vector.tensor_mul(tc_[:], tc_[:], ln1mbb[:])
                nc.vector.tensor_add(bce[:], bce[:], tc_[:])
                nc.vector.reduce_sum(acc[:, a0+1:a0+2], bce[:], axis=XY)

                # --- size term --------------------------------------------
                dsz = wk.tile([128, QT, 2], f32)
                nc.vector.tensor_sub(dsz[:], lnbb[:, :, 2:4], lngt[:])
                nc.vector.tensor_reduce(
                    acc[:, a0+2:a0+3], dsz[:], axis=XY, op=Alu.add,
                    apply_absolute_value=True)

                # --- cross-entropy ----------------------------------------
                sez = wk.tile([128, QT], f32)
                nc.vector.reduce_sum(sez[:], expz[:], axis=X)
                lnsez = wk.tile([128, QT], f32)
                nc.scalar.activation(lnsez[:], sez[:], Act.Ln)
                ced = wk.tile([128, QT], f32)
                nc.vector.tensor_sub(ced[:], z_t[:, :, 1], z_t[:, :, 0])
                nc.vector.tensor_mul(ced[:], ced[:], z_t[:, :, 2])
                nc.vector.tensor_add(ced[:], ced[:], z_t[:, :, 0])
                nc.vector.tensor_sub(ced[:], lnsez[:], ced[:])
                nc.vector.reduce_sum(acc[:, a0:a0+1], ced[:], axis=X)

            nc.sync.dma_start(out=out_d[:], in_=acc[:])

    _split_multi_waits(nc)
    return nc


def _prep_core_inputs(bbox_, bbox, cls_, cls):
    """Shard + pack host-side. Sample (mt, q, i) of a core maps to the
    core-local index mt*TILE + q*128 + i. Returns in_maps for 8 cores."""
    import ml_dtypes
    bf = ml_dtypes.bfloat16

    bbox = np.ascontiguousarray(bbox.reshape(N, 5, 49))
    bbox_ = np.ascontiguousarray(bbox_.reshape(N, 15, 49))
    probs = bbox[:, 0]                                      # [N,49] f32
    coord_idx = [1, 2, 3, 4, 6, 7, 8, 9, 11, 12, 13, 14]
    coords = np.concatenate(
        [bbox[:, 1:5], bbox_[:, coord_idx]], axis=1)        # [N,16,49]
    obj = bbox_[:, [0, 5, 10]]                              # [N,3,49]
    clsz = np.zeros((N, 4), np.float32)
    clsz[:, 0:2] = cls_
    clsz[:, 2] = cls.astype(np.float32) - 1.0

    # goff[p, j] = 49*(16j + p%16) + 48  (idxs = goff - (48 - m))
    pp = np.arange(128)[:, None] % 16
    jj = np.arange(QT)[None, :]
    goff = (49 * (16 * jj + pp) + 48).astype(np.uint16)

    maps = []
    for c in range(N_CORES):
        s = slice(c * NC_SAMP, (c + 1) * NC_SAMP)
        # views with core-local sample axis [MT, QT, 128(i)]
        def v(a):
            return a[s].reshape(MT, QT, 128, *a.shape[1:])
        cv, pv, ov, zv = v(coords), v(probs), v(obj), v(clsz)

        # cg[16q+ch, mt, i*49+cell] -- channels on partitions
        cg = np.ascontiguousarray(
            cv.transpose(1, 3, 0, 2, 4)                     # [QT,16ch,MT,128i,49]
        ).reshape(QT * 16, MT, NI * 49).astype(bf)
        # pidx[16q+v, mt, j, cell], i = 16j+v
        pidx = np.ascontiguousarray(
            pv.reshape(MT, QT, QT, 16, 49)                  # i -> (j, v)
            .transpose(1, 3, 0, 2, 4)                       # [QT,16v,MT,j,49]
        ).reshape(128, MT, QT, 49)
        # pcmp[i, mt, q, cell]
        pcmp = np.ascontiguousarray(pv.transpose(2, 0, 1, 3))
        # obj[i, mt, q, 3, 49]
        objl = np.ascontiguousarray(ov.transpose(2, 0, 1, 3, 4)).astype(bf)
        # clsz[i, mt, q, 4]
        clz = np.ascontiguousarray(zv.transpose(2, 0, 1, 3))

        maps.append({
            "cg": cg.view(np.uint16),
            "pidx": pidx,
            "pcmp": pcmp,
            "obj": objl.view(np.uint16),
            "clsz": clz,
            "goff": goff,
        })
    return maps


def _combine(results):
    parts = np.stack([r["out"] for r in results]).astype(np.float64)
    parts = parts.reshape(N_CORES, 128, MT, N_ACC)
    tot = parts.sum(axis=(0, 1, 2))        # ce, coord, size, obj, s
    ce_sum, coord_acc, size_acc, obj_acc, s_acc = tot
    total = ce_sum / N - coord_acc + size_acc + (s_acc - obj_acc) / (N * 49.0)
    return np.float32(total)


def kernel(bbox_, cls_, bbox, cls):
    global _compiled
    from concourse.bass_utils import run_bass_kernel_spmd

    bbox_ = np.asarray(bbox_, dtype=np.float32)
    bbox = np.asarray(bbox, dtype=np.float32)
    cls_ = np.asarray(cls_, dtype=np.float32)
    cls = np.asarray(cls)

    if _compiled is None:
        _compiled = _build()
    maps = _prep_core_inputs(bbox_, bbox, cls_, cls)
    res = run_bass_kernel_spmd(_compiled, maps, list(range(N_CORES)))
    return _combine(res.results)


# revision 11
# speedup vs baseline: 120.2009x; 2.4870x over previous
"""Trainium2 Bass kernel for nn_BestDetectorEverLoss.

Data-parallel over the batch dim N=65536 across 8 NeuronCores. Each core
streams its 8192 samples, computes per-sample matching / IoU / loss terms,
and reduces to per-partition partial sums; the host combines in float64.

v2 design:
  - The 16-value per-sample gather at the argmax cell runs on GpSimd via
    `indirect_copy` over a channels-on-partitions layout (each 16-partition
    group holds all 16 channels of a sample subset; per-sample cell index
    shared by the group's partitions). A PE transpose brings the gathered
    values back to samples-on-partitions.
  - The argmax chain runs in a matching "idx layout" so the uint16 index
    tile is a pure AP transform of the argmax result.
  - coords and objectness maps travel as bfloat16 (the 49-cell argmax uses
    full f32 probs, so the matched cell is exact); everything loss-critical
    is computed in f32 on-chip.
  - (cx+j)/G translation cancels in all IoU differences, so i/j are never
    computed. Anchor argmax uses first-match tie-breaking.
  - prob_loss = [sum -ln(1-p) over all anchor maps]  (ACT accum_out)
              + [sum probs*(ln(1-p_best) - ln(p_best))].
"""

import numpy as np

N_CORES = 8
N = 65536
G = 7
NC_SAMP = N // N_CORES          # 8192 samples per core
QT = 8                          # sample groups ("q") per macro-tile
TILE = 128 * QT                 # samples per macro-tile
MT = NC_SAMP // TILE            # macro-tiles per core
NI = TILE // 8                  # indices per 16-partition gather group
N_ACC = 5                       # ce, coord, size, obj, s

_compiled = None


def _split_multi_waits(nc):
    """This walrus build caps sync waits at 1 per instruction (2 for
    EventSemaphore), but Tile's sem assignment can attach several. Hoist
    extra waits onto same-engine NoOps inserted right before the
    instruction — identical blocking semantics, encodable."""
    import bass_rust

    def cap(inst):
        return 2 if isinstance(inst, bass_rust.InstEventSemaphore) else 1

    for f in nc.m.functions:
        for bb in f.blocks:
            il = bb.instructions
            i = 0
            while i < len(il):
                inst = il[i]
                si = getattr(inst, "sync_info", None)
                if si is not None and si.on_wait:
                    k = cap(inst)
                    waits = list(si.on_wait)
                    if len(waits) > k:
                        si.on_wait = waits[:k]
                        for w in waits[k:]:
                            nop = bass_rust.InstNoOp(
                                name=f"nopw-{nc.next_id()}", ins=[], outs=[])
                            nop.engine = inst.engine
                            nop.sync_info = bass_rust.SyncInfo(
                                on_wait=[w], on_update=[])
                            il.insert(i, nop)
                            i += 1
                i += 1


def _build(repeat=1):
    from concourse import bass, mybir
    from concourse.tile import TileContext

    f32 = mybir.dt.float32
    bf16 = mybir.dt.bfloat16
    u16 = mybir.dt.uint16
    i32 = mybir.dt.int32
    Alu = mybir.AluOpType
    Act = mybir.ActivationFunctionType
    X, XY, XYZ = (mybir.AxisListType.X, mybir.AxisListType.XY,
                  mybir.AxisListType.XYZ)

    nc = bass.Bass("TRN2", target_bir_lowering=False, debug=False,
                   num_devices=N_CORES)

    # DRAM inputs (per-core, host-packed layouts; see _prep_core_inputs)
    cg_d = nc.dram_tensor("cg", [128, MT, NI * 49], bf16,
                          kind="ExternalInput").ap()
    pidx_d = nc.dram_tensor("pidx", [128, MT, QT, 49], f32,
                            kind="ExternalInput").ap()
    pcmp_d = nc.dram_tensor("pcmp", [128, MT, QT, 49], f32,
                            kind="ExternalInput").ap()
    obj_d = nc.dram_tensor("obj", [128, MT, QT, 3, 49], bf16,
                           kind="ExternalInput").ap()
    clsz_d = nc.dram_tensor("clsz", [128, MT, QT, 4], f32,
                            kind="ExternalInput").ap()
    goff_d = nc.dram_tensor("goff", [128, QT], u16,
                            kind="ExternalInput").ap()
    out_d = nc.dram_tensor("out", [128, N_ACC * MT], f32,
                           kind="ExternalOutput").ap()

    with TileContext(nc) as tc:
        with tc.tile_pool(name="const", bufs=1) as cpool, \
             tc.tile_pool(name="acc", bufs=1) as apool, \
             tc.tile_pool(name="io", bufs=2) as io, \
             tc.tile_pool(name="wk", bufs=2) as wk, \
             tc.tile_pool(name="ps", bufs=2, space="PSUM") as psp:

            ones = cpool.tile([128, 1], f32)
            nc.vector.memset(ones[:], 1.0)
            # rev49[c] = 48 - c  (first-match argmax over cells)
            rev49i = cpool.tile([128, 49], i32)
            nc.gpsimd.iota(rev49i[:], pattern=[[-1, 49]], base=48,
                           channel_multiplier=0)
            rev49 = cpool.tile([128, 49], f32)
            nc.vector.tensor_copy(rev49[:], rev49i[:])
            # revk[k] = 2 - k (first-match argmax over anchors)
            revki = cpool.tile([128, 3], i32)
            nc.gpsimd.iota(revki[:], pattern=[[-1, 3]], base=2,
                           channel_multiplier=0)
            revk = cpool.tile([128, 3], f32)
            nc.vector.tensor_copy(revk[:], revki[:])
            # identity (bf16) for PE transpose
            idni = cpool.tile([128, 128], i32)
            nc.gpsimd.iota(idni[:], pattern=[[1, 128]], base=0,
                           channel_multiplier=-1)
            idn = cpool.tile([128, 128], bf16)
            nc.vector.tensor_scalar(idn[:], idni[:], 0, None, op0=Alu.is_equal)
            goff = cpool.tile([128, QT], u16)
            nc.sync.dma_start(out=goff[:], in_=goff_d[:])

            acc = apool.tile([128, N_ACC * MT], f32)

            for mt in [m for _ in range(repeat) for m in range(MT)]:
                a0 = N_ACC * mt
                cg_t = io.tile([128, NI * 49], bf16)
                pidx_t = io.tile([128, QT, 49], f32)
                pcmp_t = io.tile([128, QT, 49], f32)
                o_t = io.tile([128, QT, 3, 49], bf16)
                z_t = io.tile([128, QT, 4], f32)
                nc.sync.dma_start(out=cg_t[:], in_=cg_d[:, mt])
                nc.sync.dma_start(out=pidx_t[:], in_=pidx_d[:, mt])
                nc.sync.dma_start(out=pcmp_t[:], in_=pcmp_d[:, mt])
                nc.sync.dma_start(out=o_t[:], in_=obj_d[:, mt])
                nc.sync.dma_start(out=z_t[:], in_=clsz_d[:, mt])

                # --- argmax cell (idx layout) -----------------------------
                maxv = wk.tile([128, QT], f32)
                nc.vector.reduce_max(maxv[:], pidx_t[:], axis=X)
                eqm = wk.tile([128, QT, 49], f32)
                nc.vector.tensor_tensor(
                    eqm[:], pidx_t[:],
                    maxv[:].unsqueeze(2).broadcast_to([128, QT, 49]),
                    op=Alu.is_equal)
                mrev = wk.tile([128, QT, 49], f32)
                nc.gpsimd.tensor_tensor(
                    mrev[:], eqm[:],
                    rev49[:].unsqueeze(1).broadcast_to([128, QT, 49]),
                    op=Alu.mult)
                mx = wk.tile([128, QT], f32)
                nc.vector.reduce_max(mx[:], mrev[:], axis=X)   # = 48 - m
                mxu = wk.tile([128, QT], u16)
                nc.vector.tensor_copy(mxu[:], mx[:])
                idxs = wk.tile([128, QT], u16)
                nc.vector.tensor_sub(idxs[:], goff[:], mxu[:])  # goff = 49i+48

                # --- gather via indirect_copy + PE transpose --------------
                go = wk.tile([128, NI, 1], bf16)
                nc.gpsimd.indirect_copy(go[:], cg_t[:], idxs[:], True)
                ps = psp.tile([128, NI], bf16)
                nc.tensor.transpose(ps[:], go[:].squeeze(2), idn[:, 0:NI])
                gt = wk.tile([128, NI], f32)
                nc.scalar.copy(gt[:], ps[:])
                # gt[i, 16q+ch]; ch = 4*box + comp; boxes: gt,a0,a1,a2
                g4 = gt[:].rearrange("p (q b c) -> p q b c", b=4, c=4)

                # --- IoU (translation-invariant) --------------------------
                c7 = wk.tile([128, QT, 4, 2], f32)
                nc.vector.tensor_scalar_mul(c7[:], g4[:, :, :, 0:2], 1.0 / G)
                wh2 = wk.tile([128, QT, 4, 2], f32)
                nc.vector.tensor_scalar_mul(wh2[:], g4[:, :, :, 2:4], 0.5)
                lo_ = wk.tile([128, QT, 4, 2], f32)
                nc.vector.tensor_sub(lo_[:], c7[:], wh2[:])
                hi_ = wk.tile([128, QT, 4, 2], f32)
                nc.vector.tensor_add(hi_[:], c7[:], wh2[:])

                minhi = wk.tile([128, QT, 3, 2], f32)
                nc.vector.tensor_tensor(
                    minhi[:], hi_[:, :, 1:4, :],
                    hi_[:, :, 0:1, :].broadcast_to([128, QT, 3, 2]), op=Alu.min)
                maxlo = wk.tile([128, QT, 3, 2], f32)
                nc.vector.tensor_tensor(
                    maxlo[:], lo_[:, :, 1:4, :],
                    lo_[:, :, 0:1, :].broadcast_to([128, QT, 3, 2]), op=Alu.max)
                iwh = wk.tile([128, QT, 3, 2], f32)
                nc.vector.tensor_sub(iwh[:], minhi[:], maxlo[:])
                nc.vector.tensor_scalar_max(iwh[:], iwh[:], 0.0)

                inter = wk.tile([128, QT, 3], f32)
                nc.vector.tensor_mul(inter[:], iwh[:, :, :, 0], iwh[:, :, :, 1])
                area = wk.tile([128, QT, 4], f32)
                nc.vector.tensor_mul(area[:], g4[:, :, :, 2], g4[:, :, :, 3])
                den = wk.tile([128, QT, 3], f32)
                nc.vector.tensor_tensor(
                    den[:], area[:, :, 1:4],
                    area[:, :, 0:1].broadcast_to([128, QT, 3]), op=Alu.add)
                nc.vector.scalar_tensor_tensor(
                    den[:], inter[:], -1.0, den[:], op0=Alu.mult, op1=Alu.add)
                nc.vector.tensor_scalar_add(den[:], den[:], 1e-9)
                rden = wk.tile([128, QT, 3], f32)
                nc.vector.reciprocal(rden[:], den[:])
                iou = wk.tile([128, QT, 3], f32)
                nc.vector.tensor_mul(iou[:], inter[:], rden[:])

                # --- best anchor (first-match one-hot) --------------------
                bi = wk.tile([128, QT], f32)
                nc.vector.reduce_max(bi[:], iou[:], axis=X)
                eq3 = wk.tile([128, QT, 3], f32)
                nc.vector.tensor_tensor(
                    eq3[:], iou[:],
                    bi[:].unsqueeze(2).broadcast_to([128, QT, 3]),
                    op=Alu.is_equal)
                mrev3 = wk.tile([128, QT, 3], f32)
                nc.vector.tensor_tensor(
                    mrev3[:], eq3[:],
                    revk[:].unsqueeze(1).broadcast_to([128, QT, 3]),
                    op=Alu.mult)
                kfm = wk.tile([128, QT], f32)
                nc.vector.reduce_max(kfm[:], mrev3[:], axis=X)
                oh3 = wk.tile([128, QT, 3], f32)
                nc.vector.tensor_tensor(
                    oh3[:], revk[:].unsqueeze(1).broadcast_to([128, QT, 3]),
                    kfm[:].unsqueeze(2).broadcast_to([128, QT, 3]),
                    op=Alu.is_equal)

                # --- best box -------------------------------------------
                bprod = wk.tile([128, QT, 3, 4], f32)
                nc.vector.tensor_tensor(
                    bprod[:], g4[:, :, 1:4, :],
                    oh3[:].unsqueeze(3).broadcast_to([128, QT, 3, 4]),
                    op=Alu.mult)
                bb = wk.tile([128, QT, 4], f32)
                nc.vector.reduce_sum(bb[:], bprod[:].transpose([0, 1, 3, 2]),
                                     axis=X)

                # --- selected objectness map (GpSimd) ---------------------
                msk = wk.tile([128, QT, 3, 49], f32)
                nc.gpsimd.tensor_tensor(
                    msk[:], o_t[:],
                    oh3[:].unsqueeze(3).broadcast_to([128, QT, 3, 49]),
                    op=Alu.mult)
                ps01 = wk.tile([128, QT, 49], f32)
                nc.gpsimd.tensor_tensor(ps01[:], msk[:, :, 0, :],
                                        msk[:, :, 1, :], op=Alu.add)
                psel = wk.tile([128, QT, 49], f32)
                nc.vector.tensor_add(psel[:], ps01[:], msk[:, :, 2, :])

                # --- ACT: logs / exp --------------------------------------
                l0 = wk.tile([128, QT, 49], f32)
                nc.scalar.activation(l0[:], psel[:], Act.Ln)
                l1 = wk.tile([128, QT, 49], f32)
                nc.scalar.activation(l1[:], psel[:], Act.Ln, bias=1.0,
                                     scale=-1.0)
                l1g = wk.tile([128, QT, 3, 49], bf16)
                nc.scalar.activation(l1g[:], o_t[:], Act.Ln, bias=1.0,
                                     scale=-1.0, accum_out=acc[:, a0+3:a0+4])
                lnbb = wk.tile([128, QT, 4], f32)
                nc.scalar.activation(lnbb[:], bb[:], Act.Ln)
                ln1mbb = wk.tile([128, QT, 2], f32)
                nc.scalar.activation(ln1mbb[:], bb[:, :, 0:2], Act.Ln,
                                     bias=1.0, scale=-1.0)
                lngt = wk.tile([128, QT, 2], f32)
                nc.scalar.activation(lngt[:], g4[:, :, 0, 2:4], Act.Ln)
                expz = wk.tile([128, QT, 2], f32)
                nc.scalar.activation(expz[:], z_t[:, :, 0:2], Act.Exp)

                # --- objectness s-term ------------------------------------
                d_t = wk.tile([128, QT, 49], f32)
                nc.vector.tensor_tensor(d_t[:], l1[:], l0[:], op=Alu.subtract)
                nc.vector.tensor_tensor(d_t[:], d_t[:], pcmp_t[:], op=Alu.mult)
                nc.vector.reduce_sum(acc[:, a0+4:a0+5], d_t[:], axis=XY)

                # --- coord bce sum ----------------------------------------
                bce = wk.tile([128, QT, 2], f32)
                nc.vector.tensor_mul(bce[:], g4[:, :, 0, 0:2], lnbb[:, :, 0:2])
                tc_ = wk.tile([128, QT, 2], f32)
                nc.vector.tensor_tensor(
                    tc_[:], ones[:].unsqueeze(2).broadcast_to([128, QT, 2]),
                    g4[:, :, 0, 0:2], op=Alu.subtract)
                nc.vector.tensor_mul(tc_[:], tc_[:], ln1mbb[:])
                nc.vector.tensor_add(bce[:], bce[:], tc_[:])
                nc.vector.reduce_sum(acc[:, a0+1:a0+2], bce[:], axis=XY)

                # --- size term --------------------------------------------
                dsz = wk.tile([128, QT, 2], f32)
                nc.vector.tensor_sub(dsz[:], lnbb[:, :, 2:4], lngt[:])
                nc.vector.tensor_reduce(
                    acc[:, a0+2:a0+3], dsz[:], axis=XY, op=Alu.add,
                    apply_absolute_value=True)

                # --- cross-entropy ----------------------------------------
                sez = wk.tile([128, QT], f32)
                nc.vector.reduce_sum(sez[:], expz[:], axis=X)
                lnsez = wk.tile([128, QT], f32)
                nc.scalar.activation(lnsez[:], sez[:], Act.Ln)
                ced = wk.tile([128, QT], f32)
                nc.vector.tensor_sub(ced[:], z_t[:, :, 1], z_t[:, :, 0])
                nc.vector.tensor_mul(ced[:], ced[:], z_t[:, :, 2])
                nc.vector.tensor_add(ced[:], ced[:], z_t[:, :, 0])
                nc.vector.tensor_sub(ced[:], lnsez[:], ced[:])
                nc.vector.reduce_sum(acc[:, a0:a0+1], ced[:], axis=X)

            nc.sync.dma_start(out=out_d[:], in_=acc[:])

    _split_multi_waits(nc)
    return nc


def _prep_core_inputs(bbox_, bbox, cls_, cls):
    """Shard + pack host-side. Sample (mt, q, i) of a core maps to the
    core-local index mt*TILE + q*128 + i. Returns in_maps for 8 cores."""
    import ml_dtypes
    bf = ml_dtypes.bfloat16

    bbox = np.ascontiguousarray(bbox.reshape(N, 5, 49))
    bbox_ = np.ascontiguousarray(bbox_.reshape(N, 15, 49))
    probs = bbox[:, 0]                                      # [N,49] f32
    coord_idx = [1, 2, 3, 4, 6, 7, 8, 9, 11, 12, 13, 14]
    coords = np.concatenate(
        [bbox[:, 1:5], bbox_[:, coord_idx]], axis=1)        # [N,16,49]
    obj = bbox_[:, [0, 5, 10]]                              # [N,3,49]
    clsz = np.zeros((N, 4), np.float32)
    clsz[:, 0:2] = cls_
    clsz[:, 2] = cls.astype(np.float32) - 1.0

    # goff[p, j] = 49*(16j + p%16) + 48  (idxs = goff - (48 - m))
    pp = np.arange(128)[:, None] % 16
    jj = np.arange(QT)[None, :]
    goff = (49 * (16 * jj + pp) + 48).astype(np.uint16)

    maps = []
    for c in range(N_CORES):
        s = slice(c * NC_SAMP, (c + 1) * NC_SAMP)
        # views with core-local sample axis [MT, QT, 128(i)]
        def v(a):
            return a[s].reshape(MT, QT, 128, *a.shape[1:])
        cv, pv, ov, zv = v(coords), v(probs), v(obj), v(clsz)

        # cg[16q+ch, mt, i*49+cell] -- channels on partitions
        cg = np.ascontiguousarray(
            cv.transpose(1, 3, 0, 2, 4)                     # [QT,16ch,MT,128i,49]
        ).reshape(QT * 16, MT, NI * 49).astype(bf)
        # pidx[16q+v, mt, j, cell], i = 16j+v
        pidx = np.ascontiguousarray(
            pv.reshape(MT, QT, QT, 16, 49)                  # i -> (j, v)
            .transpose(1, 3, 0, 2, 4)                       # [QT,16v,MT,j,49]
        ).reshape(128, MT, QT, 49)
        # pcmp[i, mt, q, cell]
        pcmp = np.ascontiguousarray(pv.transpose(2, 0, 1, 3))
        # obj[i, mt, q, 3, 49]
        objl = np.ascontiguousarray(ov.transpose(2, 0, 1, 3, 4)).astype(bf)
        # clsz[i, mt, q, 4]
        clz = np.ascontiguousarray(zv.transpose(2, 0, 1, 3))

        maps.append({
            "cg": cg.view(np.uint16),
            "pidx": pidx,
            "pcmp": pcmp,
            "obj": objl.view(np.uint16),
            "clsz": clz,
            "goff": goff,
        })
    return maps


def _combine(results):
    parts = np.stack([r["out"] for r in results]).astype(np.float64)
    parts = parts.reshape(N_CORES, 128, MT, N_ACC)
    tot = parts.sum(axis=(0, 1, 2))        # ce, coord, size, obj, s
    ce_sum, coord_acc, size_acc, obj_acc, s_acc = tot
    total = ce_sum / N - coord_acc + size_acc + (s_acc - obj_acc) / (N * 49.0)
    return np.float32(total)


def kernel(bbox_, cls_, bbox, cls):
    global _compiled
    from concourse.bass_utils import run_bass_kernel_spmd

    bbox_ = np.asarray(bbox_, dtype=np.float32)
    bbox = np.asarray(bbox, dtype=np.float32)
    cls_ = np.asarray(cls_, dtype=np.float32)
    cls = np.asarray(cls)

    if _compiled is None:
        _compiled = _build()
    maps = _prep_core_inputs(bbox_, bbox, cls_, cls)
    res = run_bass_kernel_spmd(_compiled, maps, list(range(N_CORES)))
    return _combine(res.results)
